# revision 5
# baseline (speedup 1.0000x reference)
"""Trainium2 Bass kernel for nn_Head (single-head causal attention).

Contract: kernel(**inputs) takes FULL inputs (x [8,2048,1024] f32,
Wk/Wq/Wv [64,1024] f32) and returns the FULL output [8,2048,64] f32.
Data-parallel over batch B=8 across the 8 NeuronCores (one batch row per
core); each core runs an identical single-core program.

v3 design notes (engineered against v1/v2 traces):
  * The PE clock (HAM) throttles to 1.2 GHz after any ~3.4us idle window
    and that cold state dominated v1/v2 (attention ran 2x slow). Dummy
    matmuls bridge the two idle windows (DMA lead-in, projection->attention
    boundary) so the PE stays at 2.4 GHz.
  * First x chunk is DMA'd in 512-column pieces so the first projection
    matmul starts ~4us earlier; chunks stream in consumption order.
  * kq PSUM->SBUF evacuation is ONE full-width [128,512] cast per j
    (same DVE cost as half-width); the q rows then move to partition
    base 0 via tiny SBUF->SBUF DMAs (DVE cannot shift partitions).
  * v-transposes write PSUM tiles that pre-use the freed "ot" slots and
    are interleaved with the first ST tiles; vaug copies are 4 wide
    copies instead of 16 narrow ones.
  * Attention: ST tiles fp32 in PSUM, one exp per 1024-wide tile on ACT
    (the pacing engine ~21us), diagonal causal mask via gpsimd
    affine_select, PV accumulates [65,512] banks with a ones-column
    denominator row.
  * Per-bank epilogue (copy, reciprocal of denominator row, PE
    transpose, scale, per-128-row DMA out) starts as soon as a bank
    stops accumulating (i=4j+3), with the PE transposes LAGGED one
    iteration so they never head-of-line-block the in-order PE queue.
"""

import sys

if "/opt/trn_rl_repo" not in sys.path:
    sys.path.insert(0, "/opt/trn_rl_repo")

import numpy as np

B = 8
T = 2048
C = 1024
H = 64
P = 128
CB = C // P        # 8 contraction chunks
TJ = T // 512      # 4 column chunks of 512
NT = T // P        # 16 s-tiles
N_CORES = 8

_NC_CACHE = {}


def _build_nc():
    import concourse.bass as bass
    import concourse.mybir as mybir
    import concourse.tile as tile
    from concourse.bass import ts
    from concourse.masks import make_identity

    fp32 = mybir.dt.float32
    bf16 = mybir.dt.bfloat16
    EXP = mybir.ActivationFunctionType.Exp

    nc = bass.Bass(target_bir_lowering=False, debug=False)
    xt_d = nc.declare_dram_parameter("xt", [C, T], bf16, isOutput=False)
    wkq_d = nc.declare_dram_parameter("wkq", [C, P], bf16, isOutput=False)
    wv_d = nc.declare_dram_parameter("wv", [C, H], bf16, isOutput=False)
    out_d = nc.declare_dram_parameter("out", [T, H], fp32, isOutput=True)

    from contextlib import ExitStack

    with tile.TileContext(nc) as tc, ExitStack() as stk:
        pers = stk.enter_context(tc.tile_pool(name="pers", bufs=1))
        xt_sb = pers.tile([P, CB, T], bf16, tag="xt_sb", name="xt_sb")
        wkq_sb = pers.tile([P, CB, P], bf16, tag="wkq_sb", name="wkq_sb")
        wv_sb = pers.tile([P, CB, H], bf16, tag="wv_sb", name="wv_sb")
        kqt_sb = pers.tile([P, T], bf16, tag="kqt_sb", name="kqt_sb")  # k rows 0:64, q rows 64:128
        qt_sb = pers.tile([H, T], bf16, tag="qt_sb", name="qt_sb")     # q at partition base 0
        vt_sb = pers.tile([H, T], bf16, tag="vt_sb", name="vt_sb")
        vaug_sb = pers.tile([P, NT, H + 1], bf16, tag="vaug_sb", name="vaug_sb")
        ot_sb = pers.tile([H + 1, T], fp32, tag="ot_sb", name="ot_sb")
        o_sb = pers.tile([P, NT, H], fp32, tag="o_sb", name="o_sb")
        ident = pers.tile([P, P], fp32, tag="ident", name="ident")
        identb = pers.tile([H, H], bf16, tag="identb", name="identb")
        dummy_sb = pers.tile([H, P], bf16, tag="dummy_sb", name="dummy_sb")
        tl_sb = pers.tile([1, 8], fp32, tag="tl_sb", name="tl_sb")

        # ---- input DMAs first (sync HWDGE queue; arrival order matches
        # consumption: weight half 0, chunk 0 in 512-col pieces, ...) ----
        wkq_r = wkq_d.rearrange("(o p) m -> p o m", p=P)
        nc.sync.dma_start(wkq_sb[:, 0:4, :], wkq_r[:, 0:4, :])
        nc.sync.dma_start(xt_sb[:, 0, ts(0, 512)], xt_d[0:P, ts(0, 512)])
        nc.sync.dma_start(xt_sb[:, 0, ts(1, 512)], xt_d[0:P, ts(1, 512)])
        nc.sync.dma_start(wv_sb[:], wv_d.rearrange("(o p) m -> p o m", p=P))
        nc.sync.dma_start(xt_sb[:, 0, ts(2, 512)], xt_d[0:P, ts(2, 512)])
        nc.sync.dma_start(xt_sb[:, 0, ts(3, 512)], xt_d[0:P, ts(3, 512)])
        nc.sync.dma_start(wkq_sb[:, 4:8, :], wkq_r[:, 4:8, :])
        nc.sync.dma_start(xt_sb[:, 1, 0:1024], xt_d[P : 2 * P, 0:1024])
        nc.sync.dma_start(xt_sb[:, 1, 1024:2048], xt_d[P : 2 * P, 1024:2048])
        for cb in range(2, CB):
            nc.sync.dma_start(xt_sb[:, cb, :], xt_d[cb * P : (cb + 1) * P, :])

        # ---- setup constants; preload the exp table on ACT while DMA runs ----
        make_identity(nc, ident[:])
        make_identity(nc, identb[:])
        nc.gpsimd.memset(dummy_sb[:], 0.0)
        nc.gpsimd.memset(tl_sb[:], 0.0)
        nc.any.memset(vaug_sb[:, :, H], 1.0)
        nc.scalar.activation(tl_sb[:], tl_sb[:], EXP)

        # ---- HAM warm-up: dummy matmuls bridge the DMA lead-in so real
        # matmuls run at 2.4 GHz from the start ----
        with tc.tile_pool(name="warm", bufs=1, space="PSUM") as wp:
            wps = wp.tile([H, P], fp32, tag="w", name="warm")
            for _ in range(20):
                nc.tensor.matmul(wps, identb[:], dummy_sb[:], start=True, stop=True)

        # ---- projections: kqT [128, T] and vT [64, T], interleaved per chunk ----
        with tc.tile_pool(name="pp", bufs=4, space="PSUM") as pp:
            kq_ps = [pp.tile([P, 512], fp32, tag="kq", name=f"kq{j}") for j in range(TJ)]
            v_ps = [pp.tile([H, 512], fp32, tag="v", name=f"v{j}") for j in range(TJ)]
            for cb in range(CB):
                for j in range(TJ):
                    nc.tensor.matmul(
                        kq_ps[j], wkq_sb[:, cb, :], xt_sb[:, cb, ts(j, 512)],
                        start=(cb == 0), stop=(cb == CB - 1),
                    )
                for j in range(TJ):
                    nc.tensor.matmul(
                        v_ps[j], wv_sb[:, cb, :], xt_sb[:, cb, ts(j, 512)],
                        start=(cb == 0), stop=(cb == CB - 1),
                    )
            # evacuate PSUM: one full-width cast per j (k+q together), then
            # shift q rows to partition base 0 via SBUF->SBUF DMA; vt on DVE
            for j in range(TJ):
                nc.vector.tensor_copy(kqt_sb[:, ts(j, 512)], kq_ps[j])
            for j in range(TJ):
                nc.sync.dma_start(qt_sb[:, ts(j, 512)], kqt_sb[H:P, ts(j, 512)])
            for j in range(TJ):
                nc.vector.tensor_copy(vt_sb[:, ts(j, 512)], v_ps[j][:, :])

        # ---- attention ----
        out_r = out_d.rearrange("(i p) d -> p i d", p=P)
        with (
            tc.tile_pool(name="stp", bufs=2, space="PSUM") as stp,
            tc.tile_pool(name="otp", bufs=4, space="PSUM") as otp,
            tc.tile_pool(name="ptp", bufs=6) as ptp,
        ):
            # bridge the PSUM-evacuation window on the PE so the HAM clock
            # stays warm into attention (writes a scratch "st" slot; the
            # first real ST tile simply WAR-follows it on the in-order PE)
            dmt = stp.tile([P, 1024], fp32, tag="st", name="dmt")
            for _ in range(10):
                nc.tensor.matmul(dmt[0:H, 0:P], identb[:], dummy_sb[:], start=True, stop=True)

            # v natural [s, d]: PE transposes in groups of 4 into PSUM tiles
            # that pre-use the "ot" slots (free until PV_0), interleaved with
            # the first ST tiles below; one wide DVE copy per group.
            vg_ps = [otp.tile([P, 4, H], bf16, tag="ot", name=f"vg{g}") for g in range(4)]
            ot_ps = [otp.tile([H + 1, 512], fp32, tag="ot", name=f"ot{j}") for j in range(TJ)]

            def emit_vgroup(g):
                for t in range(4):
                    i = 4 * g + t
                    nc.tensor.transpose(vg_ps[g][:, t, :], vt_sb[:, ts(i, P)], identb[:])
                nc.vector.tensor_copy(vaug_sb[:, 4 * g : 4 * g + 4, 0:H], vg_ps[g])

            def emit_st(i):
                j0 = i // 4
                pts = {}
                for jj2 in range(i // 8, 2):
                    st = stp.tile([P, 1024], fp32, tag="st", name=f"st{i}_{jj2}")
                    pt = ptp.tile([P, 1024], bf16, tag="pt", name=f"pt{i}_{jj2}")
                    estart = None
                    for hh in range(2):
                        j = 2 * jj2 + hh
                        if j < j0:
                            continue
                        o = max(0, 128 * i - 512 * j)
                        lo = 512 * hh + o
                        nc.tensor.matmul(
                            st[:, lo : 512 * (hh + 1)], qt_sb[:, ts(i, P)],
                            kqt_sb[0:H, 512 * j + o : 512 * (j + 1)],
                            start=True, stop=True,
                        )
                        if estart is None:
                            estart = lo
                    nc.scalar.activation(pt[:, estart:1024], st[:, estart:1024], EXP)
                    if jj2 == i // 8:
                        # causal mask of the diagonal 128x128 block:
                        # keep pt[s, t] where t - s >= 0, else 0
                        dlo = 128 * (i % 8)
                        nc.gpsimd.affine_select(
                            out=pt[:, dlo : dlo + P],
                            in_=pt[:, dlo : dlo + P],
                            pattern=[[1, P]],
                            compare_op=mybir.AluOpType.is_ge,
                            fill=0.0,
                            base=0,
                            channel_multiplier=-1,
                        )
                    pts[jj2] = pt
                return pts

            def emit_pv(i, pts):
                j0 = i // 4
                for j in range(j0, TJ):
                    o = max(0, 128 * i - 512 * j)
                    pt = pts[j // 2]
                    lo = 512 * (j % 2) + o
                    nc.tensor.matmul(
                        ot_ps[j][:, o:512], vaug_sb[:, i, :],
                        pt[:, lo : 512 * (j % 2) + 512],
                        start=(i == 0), stop=(i == 4 * j + 3),
                    )

            pending = []

            def epilogue_copy(j):
                # bank j finished accumulating: copy out and invert the
                # denominator row once; transposes are flushed LATER so they
                # never head-of-line-block the in-order PE queue
                nc.vector.tensor_copy(ot_sb[:, ts(j, 512)], ot_ps[j])
                nc.vector.reciprocal(
                    ot_sb[H : H + 1, ts(j, 512)], ot_sb[H : H + 1, ts(j, 512)]
                )
                pending.extend(range(4 * j, 4 * j + 4))

            def flush_or(n):
                for _ in range(min(n, len(pending))):
                    ii = pending.pop(0)
                    ops = stp.tile([P, H + 1], fp32, tag="st", name=f"or{ii}")
                    nc.tensor.transpose(
                        ops, ot_sb[:, ts(ii, P)], ident[0 : H + 1, 0 : H + 1]
                    )
                    nc.vector.tensor_scalar_mul(
                        o_sb[:, ii, :], ops[:, 0:H], ops[:, H : H + 1]
                    )
                    nc.sync.dma_start(out_r[:, ii, :], o_sb[:, ii, :])

            prev = None
            for i in range(NT):
                pts = emit_st(i)
                if i < 4:
                    emit_vgroup(i)
                if prev is not None:
                    emit_pv(prev[0], prev[1])
                    if prev[0] % 4 == 3:
                        epilogue_copy(prev[0] // 4)
                flush_or(2)
                prev = (i, pts)
            emit_pv(prev[0], prev[1])
            epilogue_copy(3)
            flush_or(len(pending))

    return nc


def _split_multiwaits(nc):
    """Walrus codegen only supports one sync-wait command per instruction;
    hoist extra waits onto NoOps inserted just before (same engine queue,
    identical semantics since engines execute their queue in order)."""
    import concourse.mybir as mybir

    n = 0
    for fn in nc.m.functions:
        for block in fn.blocks:
            new_insts = []
            for inst in block.instructions:
                si = inst.sync_info
                if si is not None and si.on_wait and len(si.on_wait) > 1:
                    waits = list(si.on_wait)
                    for w in waits[:-1]:
                        n += 1
                        new_insts.append(
                            mybir.InstNoOp(
                                name=f"WH-{n}", engine=inst.engine, ins=[], outs=[],
                                sync_info=mybir.SyncInfo(on_wait=[w], on_update=[]),
                            )
                        )
                    si.on_wait = waits[-1:]
                new_insts.append(inst)
            block.instructions = new_insts
    return nc


def _get_nc():
    if "nc" not in _NC_CACHE:
        _NC_CACHE["nc"] = _split_multiwaits(_build_nc())
    return _NC_CACHE["nc"]


def _make_in_maps(x, Wk, Wq, Wv):
    import ml_dtypes

    bf16 = ml_dtypes.bfloat16
    scale = 1.0 / np.sqrt(np.float32(C))
    wkq = np.ascontiguousarray(
        np.concatenate([Wk * scale, Wq], axis=0).T.astype(bf16)
    )  # [C, 128]
    wv = np.ascontiguousarray(Wv.T.astype(bf16))  # [C, 64]
    in_maps = []
    for b in range(B):
        xt = np.ascontiguousarray(x[b].T.astype(bf16))  # [C, T]
        in_maps.append({"xt": xt, "wkq": wkq, "wv": wv})
    return in_maps


def run(x, Wk, Wq, Wv, trace=False):
    from concourse.bass_utils import run_bass_kernel_spmd

    nc = _get_nc()
    in_maps = _make_in_maps(x, Wk, Wq, Wv)
    res = run_bass_kernel_spmd(nc, in_maps, core_ids=list(range(N_CORES)), trace=trace)
    out = np.stack([np.asarray(res.results[b]["out"]) for b in range(B)], axis=0)
    return out.astype(np.float32), res


def kernel(x, Wk, Wq, Wv):
    out, _ = run(x, Wk, Wq, Wv, trace=False)
    return out


# revision 8
# speedup vs baseline: 1.0142x; 1.0142x over previous
"""Trainium2 Bass kernel for nn_Head (single-head causal attention).

Contract: kernel(**inputs) takes FULL inputs (x [8,2048,1024] f32,
Wk/Wq/Wv [64,1024] f32) and returns the FULL output [8,2048,64] f32.
Data-parallel over batch B=8 across the 8 NeuronCores (one batch row per
core); each core runs an identical single-core program.

v4 design notes (engineered against v1-v3 traces):
  * Input DMA is the projection-phase pacer. Issue all input DMAs up
    front across BOTH hardware DGE queues (sync + scalar) to maximize
    outstanding descriptors (aggregate ~390GB/s vs ~260 when serialized),
    with the first x chunk split in 512-col pieces so compute starts early.
  * The PE clock (HAM) drops to 1.2GHz after ~3.4us of idle and that cold
    state dominated v1-v3. Dummy matmuls bridge the DMA lead-in and the
    projection->attention boundary; the PSUM evacuation casts are split
    across DVE (kt j0/j1 + vt) and ACT (kt j2/j3) so the boundary is short.
  * kq evacuation: ONE full-width [128,512] cast per j (k rows 0:64 +
    q rows 64:128 together, same DVE cost as half-width); q then moves to
    partition base 0 via small SBUF->SBUF DMAs (engines cannot shift
    partitions; the tensor engine needs both operands at the same base).
  * Attention: ST tiles fp32 in PSUM, one exp per 1024-wide tile on ACT
    (the pacing engine, ~21us), diagonal causal mask via gpsimd
    affine_select, PV accumulates [65,512] banks with a ones-column
    denominator row.
  * Epilogue is transpose-free: per 512-col bank, as soon as it stops
    accumulating (i=4j+3): copy to SBUF, reciprocal of the denominator
    row, broadcast it across partitions with a K=1 ones-matmul, one
    tensor_tensor multiply, and DMA out in [H,T] layout. The final
    [T,H] transpose happens on the host (pure marshaling, like x.T on
    input). PE work for the epilogue is one tiny matmul per bank, lagged
    one iteration so it never head-of-line-blocks the in-order PE queue.
"""

import sys

if "/opt/trn_rl_repo" not in sys.path:
    sys.path.insert(0, "/opt/trn_rl_repo")

import numpy as np

B = 8
T = 2048
C = 1024
H = 64
P = 128
CB = C // P        # 8 contraction chunks
TJ = T // 512      # 4 column chunks of 512
NT = T // P        # 16 s-tiles
N_CORES = 8

_NC_CACHE = {}


def _build_nc():
    import concourse.bass as bass
    import concourse.mybir as mybir
    import concourse.tile as tile
    from concourse.bass import ts
    from concourse.masks import make_identity

    fp32 = mybir.dt.float32
    bf16 = mybir.dt.bfloat16
    EXP = mybir.ActivationFunctionType.Exp
    COPY = mybir.ActivationFunctionType.Copy

    nc = bass.Bass(target_bir_lowering=False, debug=False)
    xt_d = nc.declare_dram_parameter("xt", [C, T], bf16, isOutput=False)
    wkq_d = nc.declare_dram_parameter("wkq", [C, P], bf16, isOutput=False)
    wv_d = nc.declare_dram_parameter("wv", [C, H], bf16, isOutput=False)
    # output in [H, T] layout; host transposes to [T, H]
    out_d = nc.declare_dram_parameter("out", [H, T], fp32, isOutput=True)

    from contextlib import ExitStack

    with tile.TileContext(nc) as tc, ExitStack() as stk:
        pers = stk.enter_context(tc.tile_pool(name="pers", bufs=1))
        xt_sb = pers.tile([P, CB, T], bf16, tag="xt_sb", name="xt_sb")
        wkq_sb = pers.tile([P, CB, P], bf16, tag="wkq_sb", name="wkq_sb")
        wv_sb = pers.tile([P, CB, H], bf16, tag="wv_sb", name="wv_sb")
        kqt_sb = pers.tile([P, T], bf16, tag="kqt_sb", name="kqt_sb")  # k rows 0:64, q rows 64:128
        qt_sb = pers.tile([H, T], bf16, tag="qt_sb", name="qt_sb")     # q at partition base 0
        vt_sb = pers.tile([H, T], bf16, tag="vt_sb", name="vt_sb")
        vaug_sb = pers.tile([P, NT, H + 1], bf16, tag="vaug_sb", name="vaug_sb")
        ot_sb = pers.tile([H + 1, T], fp32, tag="ot_sb", name="ot_sb")
        o2_sb = pers.tile([H, T], fp32, tag="o2_sb", name="o2_sb")
        identb = pers.tile([H, H], bf16, tag="identb", name="identb")
        # ones row at partition 64 so it pairs with ot_sb's denominator row
        # (matmul operands must share a partition base)
        ones_sb = pers.tile([H + 1, H], fp32, tag="ones_sb", name="ones_sb")
        dummy_sb = pers.tile([H, P], bf16, tag="dummy_sb", name="dummy_sb")
        tl_sb = pers.tile([1, 8], fp32, tag="tl_sb", name="tl_sb")

        # ---- input DMAs first, spread across both HWDGE queues so many
        # descriptors are outstanding at once (higher aggregate bandwidth);
        # chunk 0 in 512-col pieces so the first projection matmul starts
        # as early as possible ----
        wkq_r = wkq_d.rearrange("(o p) m -> p o m", p=P)
        nc.sync.dma_start(wkq_sb[:, 0:4, :], wkq_r[:, 0:4, :])
        nc.scalar.dma_start(wv_sb[:], wv_d.rearrange("(o p) m -> p o m", p=P))
        nc.sync.dma_start(xt_sb[:, 0, ts(0, 512)], xt_d[0:P, ts(0, 512)])
        nc.scalar.dma_start(xt_sb[:, 0, ts(1, 512)], xt_d[0:P, ts(1, 512)])
        nc.sync.dma_start(xt_sb[:, 0, ts(2, 512)], xt_d[0:P, ts(2, 512)])
        nc.scalar.dma_start(xt_sb[:, 0, ts(3, 512)], xt_d[0:P, ts(3, 512)])
        nc.sync.dma_start(wkq_sb[:, 4:8, :], wkq_r[:, 4:8, :])
        for cb in range(1, CB):
            eng = nc.sync if cb % 2 == 1 else nc.scalar
            eng.dma_start(xt_sb[:, cb, :], xt_d[cb * P : (cb + 1) * P, :])

        # ---- setup constants; preload the exp table on ACT while DMA runs ----
        make_identity(nc, identb[:])
        nc.gpsimd.memset(ones_sb[:], 1.0)
        nc.gpsimd.memset(dummy_sb[:], 0.0)
        nc.gpsimd.memset(tl_sb[:], 0.0)
        nc.any.memset(vaug_sb[:, :, H], 1.0)
        nc.scalar.activation(tl_sb[:], tl_sb[:], EXP)

        # ---- HAM warm-up: dummy matmuls bridge the DMA lead-in so real
        # matmuls run at 2.4 GHz from the start ----
        with tc.tile_pool(name="warm", bufs=1, space="PSUM") as wp:
            wps = wp.tile([H, P], fp32, tag="w", name="warm")
            for _ in range(20):
                nc.tensor.matmul(wps, identb[:], dummy_sb[:], start=True, stop=True)

        # ---- projections: kqT [128, T] and vT [64, T], interleaved per chunk ----
        with tc.tile_pool(name="pp", bufs=4, space="PSUM") as pp:
            kq_ps = [pp.tile([P, 512], fp32, tag="kq", name=f"kq{j}") for j in range(TJ)]
            v_ps = [pp.tile([H, 512], fp32, tag="v", name=f"v{j}") for j in range(TJ)]
            for cb in range(CB):
                for j in range(TJ):
                    nc.tensor.matmul(
                        kq_ps[j], wkq_sb[:, cb, :], xt_sb[:, cb, ts(j, 512)],
                        start=(cb == 0), stop=(cb == CB - 1),
                    )
                for j in range(TJ):
                    nc.tensor.matmul(
                        v_ps[j], wv_sb[:, cb, :], xt_sb[:, cb, ts(j, 512)],
                        start=(cb == 0), stop=(cb == CB - 1),
                    )

            # evacuate PSUM: one full-width cast per j (k+q together), split
            # across DVE (j0/j1) and ACT (j2/j3) so the boundary is short;
            # then shift q rows to partition base 0 via SBUF->SBUF DMA
            nc.vector.tensor_copy(kqt_sb[:, ts(0, 512)], kq_ps[0])
            nc.scalar.activation(kqt_sb[:, ts(2, 512)], kq_ps[2], COPY)
            nc.vector.tensor_copy(kqt_sb[:, ts(1, 512)], kq_ps[1])
            nc.scalar.activation(kqt_sb[:, ts(3, 512)], kq_ps[3], COPY)
            for j in range(TJ):
                nc.sync.dma_start(qt_sb[:, ts(j, 512)], kqt_sb[H:P, ts(j, 512)])
            for j in range(TJ):
                nc.vector.tensor_copy(vt_sb[:, ts(j, 512)], v_ps[j][:, :])

        # ---- attention ----
        with (
            tc.tile_pool(name="stp", bufs=2, space="PSUM") as stp,
            tc.tile_pool(name="otp", bufs=4, space="PSUM") as otp,
            tc.tile_pool(name="ptp", bufs=6) as ptp,
        ):
            # bridge the PSUM-evacuation window on the PE so the HAM clock
            # stays warm into attention (scratch "st" slot; the first real
            # ST tile simply WAR-follows on the in-order PE queue)
            dmt = stp.tile([P, 1024], fp32, tag="st", name="dmt")
            for _ in range(14):
                nc.tensor.matmul(dmt[0:H, 0:P], identb[:], dummy_sb[:], start=True, stop=True)

            # v natural [s, d]: PE transposes in groups of 4 into PSUM tiles
            # that pre-use the "ot" slots (free until PV_0), interleaved with
            # the first ST tiles; one wide DVE copy per group. The "ot" tag
            # rotation is vg0-3 -> ot0-3 -> rb0-3, each reusing the slot the
            # moment its predecessor is released.
            vg_ps = [otp.tile([P, 4, H], bf16, tag="ot", name=f"vg{g}") for g in range(4)]
            ot_ps = [otp.tile([H + 1, 512], fp32, tag="ot", name=f"ot{j}") for j in range(TJ)]

            def emit_vgroup(g):
                for t in range(4):
                    i = 4 * g + t
                    nc.tensor.transpose(vg_ps[g][:, t, :], vt_sb[:, ts(i, P)], identb[:])
                nc.vector.tensor_copy(vaug_sb[:, 4 * g : 4 * g + 4, 0:H], vg_ps[g])

            def emit_st(i):
                j0 = i // 4
                pts = {}
                for jj2 in range(i // 8, 2):
                    st = stp.tile([P, 1024], fp32, tag="st", name=f"st{i}_{jj2}")
                    pt = ptp.tile([P, 1024], bf16, tag="pt", name=f"pt{i}_{jj2}")
                    estart = None
                    for hh in range(2):
                        j = 2 * jj2 + hh
                        if j < j0:
                            continue
                        o = max(0, 128 * i - 512 * j)
                        lo = 512 * hh + o
                        nc.tensor.matmul(
                            st[:, lo : 512 * (hh + 1)], qt_sb[:, ts(i, P)],
                            kqt_sb[0:H, 512 * j + o : 512 * (j + 1)],
                            start=True, stop=True,
                        )
                        if estart is None:
                            estart = lo
                    nc.scalar.activation(pt[:, estart:1024], st[:, estart:1024], EXP)
                    if jj2 == i // 8:
                        # causal mask of the diagonal 128x128 block:
                        # keep pt[s, t] where t - s >= 0, else 0
                        dlo = 128 * (i % 8)
                        nc.gpsimd.affine_select(
                            out=pt[:, dlo : dlo + P],
                            in_=pt[:, dlo : dlo + P],
                            pattern=[[1, P]],
                            compare_op=mybir.AluOpType.is_ge,
                            fill=0.0,
                            base=0,
                            channel_multiplier=-1,
                        )
                    pts[jj2] = pt
                return pts

            def emit_pv(i, pts):
                j0 = i // 4
                for j in range(j0, TJ):
                    o = max(0, 128 * i - 512 * j)
                    pt = pts[j // 2]
                    lo = 512 * (j % 2) + o
                    nc.tensor.matmul(
                        ot_ps[j][:, o:512], vaug_sb[:, i, :],
                        pt[:, lo : 512 * (j % 2) + 512],
                        start=(i == 0), stop=(i == 4 * j + 3),
                    )

            pending = []

            def epilogue_copy(j):
                # bank j finished accumulating: copy out, invert the
                # denominator row; the normalize itself is flushed later so
                # its PE matmul never head-of-line-blocks the PE queue
                nc.vector.tensor_copy(ot_sb[:, ts(j, 512)], ot_ps[j])
                nc.vector.reciprocal(
                    ot_sb[H : H + 1, ts(j, 512)], ot_sb[H : H + 1, ts(j, 512)]
                )
                pending.append(j)

            def flush_norm():
                for j in pending:
                    rb = otp.tile([H, 512], fp32, tag="ot", name=f"rb{j}")
                    # broadcast 1/denom across partitions with a K=1 matmul
                    nc.tensor.matmul(
                        rb, ones_sb[H : H + 1, :], ot_sb[H : H + 1, ts(j, 512)],
                        start=True, stop=True,
                    )
                    nc.vector.tensor_tensor(
                        o2_sb[:, ts(j, 512)], ot_sb[0:H, ts(j, 512)], rb,
                        mybir.AluOpType.mult,
                    )
                    eng = nc.sync if j % 2 == 0 else nc.scalar
                    eng.dma_start(out_d[:, ts(j, 512)], o2_sb[:, ts(j, 512)])
                pending.clear()

            prev = None
            for i in range(NT):
                pts = emit_st(i)
                if i < 4:
                    emit_vgroup(i)
                if prev is not None:
                    emit_pv(prev[0], prev[1])
                    if prev[0] % 4 == 3:
                        epilogue_copy(prev[0] // 4)
                flush_norm()
                prev = (i, pts)
            emit_pv(prev[0], prev[1])
            epilogue_copy(3)
            flush_norm()

    return nc


def _split_multiwaits(nc):
    """Walrus codegen only supports one sync-wait command per instruction;
    hoist extra waits onto NoOps inserted just before (same engine queue,
    identical semantics since engines execute their queue in order)."""
    import concourse.mybir as mybir

    n = 0
    for fn in nc.m.functions:
        for block in fn.blocks:
            new_insts = []
            for inst in block.instructions:
                si = inst.sync_info
                if si is not None and si.on_wait and len(si.on_wait) > 1:
                    waits = list(si.on_wait)
                    for w in waits[:-1]:
                        n += 1
                        new_insts.append(
                            mybir.InstNoOp(
                                name=f"WH-{n}", engine=inst.engine, ins=[], outs=[],
                                sync_info=mybir.SyncInfo(on_wait=[w], on_update=[]),
                            )
                        )
                    si.on_wait = waits[-1:]
                new_insts.append(inst)
            block.instructions = new_insts
    return nc


def _get_nc():
    if "nc" not in _NC_CACHE:
        _NC_CACHE["nc"] = _split_multiwaits(_build_nc())
    return _NC_CACHE["nc"]


def _make_in_maps(x, Wk, Wq, Wv):
    import ml_dtypes

    bf16 = ml_dtypes.bfloat16
    scale = 1.0 / np.sqrt(np.float32(C))
    wkq = np.ascontiguousarray(
        np.concatenate([Wk * scale, Wq], axis=0).T.astype(bf16)
    )  # [C, 128]
    wv = np.ascontiguousarray(Wv.T.astype(bf16))  # [C, 64]
    in_maps = []
    for b in range(B):
        xt = np.ascontiguousarray(x[b].T.astype(bf16))  # [C, T]
        in_maps.append({"xt": xt, "wkq": wkq, "wv": wv})
    return in_maps


def run(x, Wk, Wq, Wv, trace=False):
    from concourse.bass_utils import run_bass_kernel_spmd

    nc = _get_nc()
    in_maps = _make_in_maps(x, Wk, Wq, Wv)
    res = run_bass_kernel_spmd(nc, in_maps, core_ids=list(range(N_CORES)), trace=trace)
    # device output is [H, T]; transpose back to [T, H] per batch row
    out = np.stack(
        [np.asarray(res.results[b]["out"]).T for b in range(B)], axis=0
    )
    return np.ascontiguousarray(out).astype(np.float32), res


def kernel(x, Wk, Wq, Wv):
    out, _ = run(x, Wk, Wq, Wv, trace=False)
    return out
